# revision 1
# baseline (speedup 1.0000x reference)
"""Trainium2 Bass kernel for CostAwareHeteroMoE.

Strategy: data-parallel over tokens across 8 NeuronCores (1024 tokens/core),
all weights replicated, no collectives. Activations are kept feature-major
([features-on-partitions, tokens-on-free]) so every matmul chains without
transposes; per-token routing weights are applied along the free dim via a
broadcast tile built on-chip.

Math rewrite (validated vs reference at ~3e-7 rel err):
  out = (sum_e W[:,e] * (gelu(gelu(h) @ w1_e + b1_e) @ w2_e + b2'_e)
         + 0.1 * gelu(gelu(h) @ sw1 + sb1) @ sw2 + const) @ up_w + up_b'
        + gelu(x) @ core_w + core_b
where W[:,e] are dense top-2 routing weights (0 elsewhere), b2'_e = b2_e - c_e
folds the "bias leak" of unrouted tokens (c_e = gelu(b1_e) @ w2_e + b2_e,
which reference adds for every unrouted token), and up_b' absorbs the
constant  (sum_e c_e + 0.1 * sb2) @ up_w.
"""

import sys

import numpy as np

sys.path.insert(0, "/opt/trn_rl_repo")

B, T, D, L = 4, 2048, 2048, 1024
HID = [1024, 2048, 3072, 4096, 1024, 2048, 3072, 4096]
E = 8
TOP_K = 2
COST_LAMBDA = 1e-7
NCORES = 8
NTOK = B * T
TPC = NTOK // NCORES  # 1024 tokens per core
P = 128
HGRP = 512  # expert hidden rows per weight-slice group


def _gelu_np(v):
    from scipy.special import erf

    return 0.5 * v * (1.0 + erf(v / np.sqrt(2.0)))


def _build_program():
    import concourse.bass as bass
    from concourse import bacc
    import concourse.mybir as mybir
    import concourse.tile as tile
    from concourse.masks import make_identity

    f32 = mybir.dt.float32
    f32r = mybir.dt.float32r
    AF = mybir.ActivationFunctionType
    ALU = mybir.AluOpType
    AX = mybir.AxisListType

    def r(ap):  # operands are already fp32r-typed
        return ap

    nc = bacc.Bacc("TRN2", debug=False)

    # ---- DRAM I/O ----
    xt = nc.dram_tensor("xt", [D, TPC], f32r, kind="ExternalInput").ap()
    dw = nc.dram_tensor("dw", [D, L], f32r, kind="ExternalInput").ap()
    rw = nc.dram_tensor("rw", [D, E], f32r, kind="ExternalInput").ap()
    upw = nc.dram_tensor("upw", [L, D], f32r, kind="ExternalInput").ap()
    corw = nc.dram_tensor("corw", [D, D], f32r, kind="ExternalInput").ap()
    sw1 = nc.dram_tensor("sw1", [L, L], f32r, kind="ExternalInput").ap()
    sw2 = nc.dram_tensor("sw2", [L, L], f32r, kind="ExternalInput").ap()  # pre-scaled by 0.1
    ew1 = [nc.dram_tensor(f"e{e}w1", [L, HID[e]], f32r, kind="ExternalInput").ap() for e in range(E)]
    ew2 = [nc.dram_tensor(f"e{e}w2", [HID[e], L], f32r, kind="ExternalInput").ap() for e in range(E)]
    # biases, host-prelaid into per-partition layouts
    dbpp = nc.dram_tensor("dbpp", [P, L // P], f32, kind="ExternalInput").ap()
    sb1pp = nc.dram_tensor("sb1pp", [P, L // P], f32, kind="ExternalInput").ap()
    b1pp = [nc.dram_tensor(f"b1pp{e}", [P, HID[e] // P], f32, kind="ExternalInput").ap() for e in range(E)]
    b2mat = nc.dram_tensor("b2mat", [E, L], f32r, kind="ExternalInput").ap()
    obias = nc.dram_tensor("obias", [1, D], f32r, kind="ExternalInput").ap()
    rbias = nc.dram_tensor("rbias", [1, E], f32r, kind="ExternalInput").ap()
    onesv = nc.dram_tensor("onesv", [1, 512], f32r, kind="ExternalInput").ap()
    out = nc.dram_tensor("out", [D, TPC], f32, kind="ExternalOutput").ap()
    wt_dram = nc.dram_tensor("wt_scratch", [E, TPC], f32r).ap()  # internal scratch

    KD = D // P      # 16 k-chunks over D
    KL = L // P      # 8 k-chunks over L
    NH = TPC // 512  # 2 token halves of 512

    with tile.TileContext(nc) as tc:
        import contextlib

        with contextlib.ExitStack() as ctx:
            const = ctx.enter_context(tc.tile_pool(name="const", bufs=1))
            gpool = ctx.enter_context(tc.tile_pool(name="gpool", bufs=1))
            psum = ctx.enter_context(tc.tile_pool(name="psum", bufs=2, space="PSUM"))

            ident = const.tile([P, P], f32)
            make_identity(nc, ident)
            ones = const.tile([1, 512], f32r)
            nc.sync.dma_start(ones, onesv)

            rw_sb = const.tile([P, KD, E], f32r)
            nc.sync.dma_start(rw_sb, rw.rearrange("(ko ki) e -> ki ko e", ki=P))
            rb_sb = const.tile([1, E], f32r)
            nc.sync.dma_start(rb_sb, rbias)
            db_sb = const.tile([P, L // P], f32)
            nc.sync.dma_start(db_sb, dbpp)
            sb1_sb = const.tile([P, L // P], f32)
            nc.sync.dma_start(sb1_sb, sb1pp)
            b1_sb = []
            for e in range(E):
                t_ = const.tile([P, HID[e] // P], f32, tag=f"b1sb{e}")
                nc.sync.dma_start(t_, b1pp[e])
                b1_sb.append(t_)
            b2_sb = const.tile([E, L], f32r)
            nc.sync.dma_start(b2_sb, b2mat)
            ob_sb = const.tile([1, D], f32r)
            nc.sync.dma_start(ob_sb, obias)
            wt_sb = const.tile([E, TPC], f32r)  # routing weights, feature-major [E, tokens]

            g = gpool.tile([P, KL, TPC], f32r)   # gelu(h), feature-major
            y = gpool.tile([P, KL, TPC], f32r)   # pre-up accumulator (first written by b2 pass)

            xt3 = xt.rearrange("(ko ki) t -> ki ko t", ki=P)

            # ============ Stage A+B: router + down-projection ============
            with contextlib.ExitStack() as sab:
                xpool = sab.enter_context(tc.tile_pool(name="xpool", bufs=2))
                dwp = sab.enter_context(tc.tile_pool(name="dwp", bufs=2))
                rwork = sab.enter_context(tc.tile_pool(name="rwork", bufs=4))
                rpsum = sab.enter_context(tc.tile_pool(name="rpsum", bufs=2, space="PSUM"))
                tpsum = sab.enter_context(tc.tile_pool(name="tpsum", bufs=2, space="PSUM"))

                for half in range(NH):
                    ts_ = slice(half * 512, (half + 1) * 512)
                    xth = xpool.tile([P, KD, 512], f32r, tag="xth")
                    nc.sync.dma_start(xth, xt3[:, :, ts_])

                    # ---- router on this half's 4 token-chunks of 128 ----
                    for tj in range(4):
                        t0 = half * 512 + tj * 128
                        rp = rpsum.tile([P, E], f32, tag="rp")
                        for k in range(KD):
                            nc.tensor.matmul(
                                rp, r(xth[:, k, tj * 128:(tj + 1) * 128]), r(rw_sb[:, k, :]),
                                start=(k == 0), stop=False,
                            )
                        nc.tensor.matmul(rp, r(ones[:, :P]), r(rb_sb), start=False, stop=True)
                        nmax = rwork.tile([P, 1], f32, tag="nmax")
                        nc.vector.tensor_reduce(nmax, rp, axis=AX.X, op=ALU.max, negate=True)
                        pexp = rwork.tile([P, E], f32, tag="pexp")
                        nc.scalar.activation(pexp, rp, AF.Exp, bias=nmax)
                        ssum = rwork.tile([P, 1], f32, tag="ssum")
                        nc.vector.tensor_reduce(ssum, pexp, axis=AX.X, op=ALU.add)
                        rs = rwork.tile([P, 1], f32, tag="rs")
                        nc.vector.reciprocal(rs, ssum)
                        probs = rwork.tile([P, E], f32, tag="probs")
                        nc.vector.tensor_scalar_mul(probs, pexp, rs)
                        p1 = rwork.tile([P, 1], f32, tag="p1")
                        nc.vector.tensor_reduce(p1, probs, axis=AX.X, op=ALU.max)
                        mlt = rwork.tile([P, E], f32, tag="mlt")
                        nc.vector.tensor_scalar(mlt, probs, p1, None, op0=ALU.is_lt)
                        pz = rwork.tile([P, E], f32, tag="pz")
                        nc.vector.tensor_mul(pz, probs, mlt)
                        p2 = rwork.tile([P, 1], f32, tag="p2")
                        nc.vector.tensor_reduce(p2, pz, axis=AX.X, op=ALU.max)
                        dd = rwork.tile([P, 1], f32, tag="dd")
                        nc.vector.tensor_scalar(dd, p2, p1, None, op0=ALU.subtract)
                        s2 = rwork.tile([P, 1], f32, tag="s2")
                        nc.scalar.activation(s2, dd, AF.Sigmoid)
                        s1 = rwork.tile([P, 1], f32, tag="s1")
                        nc.vector.tensor_scalar(s1, s2, -1.0, 1.0, op0=ALU.mult, op1=ALU.add)
                        m1 = rwork.tile([P, E], f32, tag="m1")
                        nc.vector.tensor_scalar(m1, probs, p1, None, op0=ALU.is_ge)
                        m2 = rwork.tile([P, E], f32, tag="m2")
                        nc.vector.tensor_scalar(m2, pz, p2, None, op0=ALU.is_ge)
                        wc1 = rwork.tile([P, E], f32, tag="wc1")
                        nc.vector.tensor_scalar_mul(wc1, m1, s1)
                        wc = rwork.tile([P, E], f32, tag="wc")
                        nc.vector.tensor_scalar_mul(wc, m2, s2)
                        nc.vector.tensor_add(wc, wc, wc1)
                        # transpose [128 tok, E] -> [E, 128 tok] into wt_sb
                        tp = tpsum.tile([E, P], f32, tag="tp")
                        nc.tensor.transpose(tp, wc, ident)
                        nc.vector.tensor_copy(wt_sb[:, t0:t0 + 128], tp)
                        nc.sync.dma_start(wt_dram[:, t0:t0 + 128], wt_sb[:, t0:t0 + 128])

                    # ---- down-projection for this half ----
                    for m in range(KL):
                        dsl = dwp.tile([P, KD, P], f32r, tag="dsl")
                        nc.sync.dma_start(
                            dsl, dw.rearrange("(ko ki) l -> ki ko l", ki=P)[:, :, m * P:(m + 1) * P]
                        )
                        hp = psum.tile([P, 512], f32, tag="a")
                        for k in range(KD):
                            nc.tensor.matmul(
                                hp, r(dsl[:, k, :]), r(xth[:, k, :]),
                                start=(k == 0), stop=(k == KD - 1),
                            )
                        nc.scalar.activation(g[:, m, ts_], hp, AF.Gelu, bias=db_sb[:, m:m + 1])

            # ============ Stage C: experts (+ shared, + b2 correction) ============
            with contextlib.ExitStack() as sex:
                wbp = sex.enter_context(tc.tile_pool(name="wbp", bufs=1))
                wep = sex.enter_context(tc.tile_pool(name="wep", bufs=2))
                ework = sex.enter_context(tc.tile_pool(name="ework", bufs=8))
                gawork = sex.enter_context(tc.tile_pool(name="gawork", bufs=3))

                # broadcast routing weights to all partitions: Wb[p, e, t] = W[t, e]
                wb = wbp.tile([P, E, TPC], f32r)
                nc.sync.dma_start(wb, wt_dram.partition_broadcast(P))

                # b2' correction initializes y: y = W @ b2mat   (K=E matmul)
                for m in range(KL):
                    for half in range(NH):
                        ts_ = slice(half * 512, (half + 1) * 512)
                        yp = psum.tile([P, 512], f32, tag="y")
                        nc.tensor.matmul(
                            yp, r(b2_sb[:, m * P:(m + 1) * P]), r(wt_sb[:, ts_]), start=True, stop=True
                        )
                        nc.vector.tensor_copy(y[:, m, ts_], yp)

                def mlp_block(w1_ap, w2_ap, h_dim, b1_tile, scale_e):
                    """y += [Wb_e *] gelu(w1.T@g + b1) via w2, streamed in HGRP row groups."""
                    for gi in range(h_dim // HGRP):
                        w1s = wep.tile([P, KL, HGRP], f32r, tag="w1s")
                        nc.sync.dma_start(
                            w1s,
                            w1_ap.rearrange("(ko ki) h -> ki ko h", ki=P)[:, :, gi * HGRP:(gi + 1) * HGRP],
                        )
                        w2s = wep.tile([P, HGRP // P, L], f32r, tag="w2s")
                        nc.sync.dma_start(
                            w2s,
                            w2_ap.rearrange("(ko ki) l -> ki ko l", ki=P)[:, gi * (HGRP // P):(gi + 1) * (HGRP // P), :],
                        )
                        for half in range(NH):
                            ts_ = slice(half * 512, (half + 1) * 512)
                            sga = []
                            for hc in range(HGRP // P):
                                ap_ = psum.tile([P, 512], f32, tag="a")
                                for k in range(KL):
                                    nc.tensor.matmul(
                                        ap_, r(w1s[:, k, hc * P:(hc + 1) * P]), r(g[:, k, ts_]),
                                        start=(k == 0), stop=(k == KL - 1),
                                    )
                                ga = gawork.tile([P, 512], f32r, tag="ga")
                                nc.scalar.activation(
                                    ga, ap_, AF.Gelu,
                                    bias=b1_tile[:, gi * (HGRP // P) + hc: gi * (HGRP // P) + hc + 1],
                                )
                                sg = ework.tile([P, 512], f32r, tag="sga")
                                if scale_e is not None:
                                    nc.vector.tensor_mul(sg, ga, wb[:, scale_e, ts_])
                                else:
                                    nc.vector.tensor_copy(sg, ga)
                                sga.append(sg)
                            for m in range(KL):
                                yp = psum.tile([P, 512], f32, tag="y")
                                for hc in range(HGRP // P):
                                    nc.tensor.matmul(
                                        yp, r(w2s[:, hc, m * P:(m + 1) * P]), r(sga[hc]),
                                        start=(hc == 0), stop=(hc == HGRP // P - 1),
                                    )
                                nc.vector.tensor_add(y[:, m, ts_], y[:, m, ts_], yp)

                for e in range(E):
                    mlp_block(ew1[e], ew2[e], HID[e], b1_sb[e], e)
                mlp_block(sw1, sw2, L, sb1_sb, None)  # shared branch (w2 pre-scaled 0.1)

            # ============ Stage E: up-projection + core branch ============
            with contextlib.ExitStack() as se:
                gxp = se.enter_context(tc.tile_pool(name="gxp", bufs=1))
                stg = se.enter_context(tc.tile_pool(name="stg", bufs=2))
                wup = se.enter_context(tc.tile_pool(name="wup", bufs=2))
                otp = se.enter_context(tc.tile_pool(name="otp", bufs=3))

                gx = gxp.tile([P, KD, TPC], f32r)
                for k in range(KD):
                    st_ = stg.tile([P, TPC], f32r, tag="st")
                    nc.sync.dma_start(st_, xt3[:, k, :])
                    nc.scalar.activation(gx[:, k, :], st_, AF.Gelu)

                for m in range(KD):
                    ms = slice(m * P, (m + 1) * P)
                    usl = wup.tile([P, KL, P], f32r, tag="usl")
                    nc.sync.dma_start(usl, upw.rearrange("(ko ki) d -> ki ko d", ki=P)[:, :, ms])
                    csl = wup.tile([P, KD, P], f32r, tag="csl")
                    nc.sync.dma_start(csl, corw.rearrange("(ko ki) d -> ki ko d", ki=P)[:, :, ms])
                    for half in range(NH):
                        ts_ = slice(half * 512, (half + 1) * 512)
                        op_ = psum.tile([P, 512], f32, tag="a")
                        for k in range(KL):
                            nc.tensor.matmul(op_, r(usl[:, k, :]), r(y[:, k, ts_]), start=(k == 0), stop=False)
                        for k in range(KD):
                            nc.tensor.matmul(op_, r(csl[:, k, :]), r(gx[:, k, ts_]), start=False, stop=False)
                        nc.tensor.matmul(op_, r(ob_sb[:1, ms]), r(ones[:1, :512]), start=False, stop=True)
                        ot = otp.tile([P, 512], f32, tag="ot")
                        nc.vector.tensor_copy(ot, op_)
                        nc.sync.dma_start(out[ms, ts_], ot)

    nc.finalize()
    return nc


def kernel(**inputs):
    from concourse.bass_utils import run_bass_kernel_spmd

    inp = {k: np.ascontiguousarray(np.asarray(v, dtype=np.float32)) for k, v in inputs.items()}
    x = inp["x"].reshape(NTOK, D)

    # ---- host-side weight preprocessing (pure layout/folding, no token math) ----
    cost = np.array([2 * L * h for h in HID], np.float32)
    rbias = (inp["router_b"] - COST_LAMBDA * cost).reshape(1, E)
    c = [
        _gelu_np(inp[f"e{e}_b1"]) @ inp[f"e{e}_w2"] + inp[f"e{e}_b2"]
        for e in range(E)
    ]
    b2mat = np.stack([inp[f"e{e}_b2"] - c[e] for e in range(E)], axis=0)  # [E, L]
    const_l = np.sum(c, axis=0) + 0.1 * inp["shared_b2"]
    obias = (inp["up_b"] + const_l @ inp["up_w"] + inp["core_b"]).reshape(1, D)

    common = {
        "dw": inp["down_w"],
        "rw": inp["router_w"],
        "upw": inp["up_w"],
        "corw": inp["core_w"],
        "sw1": inp["shared_w1"],
        "sw2": np.ascontiguousarray(0.1 * inp["shared_w2"]),
        "dbpp": np.ascontiguousarray(inp["down_b"].reshape(L // P, P).T),
        "sb1pp": np.ascontiguousarray(inp["shared_b1"].reshape(L // P, P).T),
        "b2mat": np.ascontiguousarray(b2mat),
        "obias": np.ascontiguousarray(obias),
        "rbias": np.ascontiguousarray(rbias),
        "onesv": np.ones((1, 512), np.float32),
    }
    for e in range(E):
        common[f"e{e}w1"] = inp[f"e{e}_w1"]
        common[f"e{e}w2"] = inp[f"e{e}_w2"]
        common[f"b1pp{e}"] = np.ascontiguousarray(inp[f"e{e}_b1"].reshape(HID[e] // P, P).T)

    in_maps = []
    for cidx in range(NCORES):
        m = dict(common)
        m["xt"] = np.ascontiguousarray(x[cidx * TPC:(cidx + 1) * TPC].T)
        in_maps.append(m)

    nc = _build_program()
    res = run_bass_kernel_spmd(nc, in_maps, list(range(NCORES)))

    full = np.empty((NTOK, D), np.float32)
    for cidx in range(NCORES):
        full[cidx * TPC:(cidx + 1) * TPC] = res.results[cidx]["out"].T
    return full.reshape(B, T, D)

